# revision 6
# baseline (speedup 1.0000x reference)
"""Trainium2 Bass kernel for nn_CrossAttention (efficient-attention form).

Reference computation per batch b:
    K = softmax(x2, axis=-1)           # over D
    Q = softmax(x2, axis=1)            # over N
    out = ((x @ K.T) @ Q) @ W.T + b

Reassociated (matmuls are associative; both softmaxes share exp(x2)):
    E  = exp(x2)                       # one exp serves both softmaxes
    K  = E * (1/rowsum(E))             # per-row scale
    M''^T = E^T K                      # accumulated in PSUM over the x2 stream
    s  = 1/colsum(E) = 1/rowsum(M''^T) # rows of K sum to 1
    C  = M'' diag(s) W^T               # [D, D]
    out = x @ C + b                    # single [N,D]@[D,D] matmul on x

Batch dim B=8 is data-parallel across the 8 cores.

v2 layout/engine plan (from trace analysis of the 75.7us baseline):
  - All transposes (x^T for the final matmul, W^T) moved OFF the tensor
    engine onto the DMA XBAR (dma_start_transpose, bf16): the PE now only
    runs the three real matmul stages (64+16+64 instructions).
  - Inputs stream on ONE sync-engine queue in priority order
    x2 (g0/g1 split per-t for an early exp start) -> W -> x, so x2 never
    loses HBM bandwidth to x; transposes issue on sync after the inputs.
  - fp32->bf16 casts: x on GpSimd (otherwise idle), W on ACT (idle after
    the exp stream). DVE keeps softmax scales, normalize, and bias adds.
  - PSUM: 4 banks for the E^T K accumulator + a 4-buf pool shared by the
    C-phase and out-phase tiles (the baseline had only 2 out banks).
  - Whole M''/C chain in bf16 (measured: bf16 and f32r matmuls stream at
    the same 1 cycle/row, but bf16 enables the DMA-transpose path).
"""

import sys

import numpy as np

if "/opt/trn_rl_repo" not in sys.path:
    sys.path.insert(0, "/opt/trn_rl_repo")

import concourse.bass as bass
import concourse.mybir as mybir
import concourse.tile as tile
from concourse import bacc
from concourse.bass import ds, ts
from concourse.bass_utils import run_bass_kernel_spmd

B, N, D = 8, 2048, 512
P = 128
T = 2             # rows per partition per group
G = N // (P * T)  # 8 row groups
DC = D // P       # 4 column chunks of D
F32 = mybir.dt.float32
BF16 = mybir.dt.bfloat16

_CACHE = {}


def _build_nc():
    nc = bacc.Bacc("TRN2", target_bir_lowering=False, debug=False)
    x_d = nc.declare_dram_parameter("x", [N, D], F32, isOutput=False)
    x2_d = nc.declare_dram_parameter("x2", [N, D], F32, isOutput=False)
    w_d = nc.declare_dram_parameter("W", [D, D], F32, isOutput=False)
    b_d = nc.declare_dram_parameter("b", [D], F32, isOutput=False)
    out_d = nc.declare_dram_parameter("out", [N, D], F32, isOutput=True)

    # input row n = g*256 + p*2 + t -> per-partition DRAM span is 2 rows = 4KB
    x_t = x_d[:].rearrange("(g p t) d -> g p t d", p=P, t=T)
    x2_t = x2_d[:].rearrange("(g p t) d -> g p t d", p=P, t=T)
    w_t = w_d[:].rearrange("(j p) d -> p j d", p=P)
    # the DMA XBAR transpose hands psum partitions back in the same
    # (g p t) row order, so outputs store with the input layout
    out_t = out_d[:].rearrange("(g p t) d -> g p t d", p=P, t=T)

    with tile.TileContext(nc) as tc:
        with (
            tc.tile_pool(name="big", bufs=1) as big,
            tc.tile_pool(name="x2s", bufs=G) as x2s,
            tc.tile_pool(name="xs", bufs=G) as xs,
            tc.tile_pool(name="small", bufs=1) as small,
            tc.tile_pool(name="stats", bufs=4) as stats,
            tc.tile_pool(name="outp", bufs=2) as outp,
            tc.tile_pool(name="psA", bufs=1, space="PSUM") as psA,
            tc.tile_pool(name="psO", bufs=4, space="PSUM") as psO,
        ):
            # ---- persistent SBUF tensors
            e_all = big.tile([P, G, T, D], BF16, tag="e_all")    # exp(x2)
            k_all = big.tile([P, G, T, D], BF16, tag="k_all")    # K rows
            xb_all = big.tile([P, G, T * D], BF16, tag="xb_all")  # x cast
            # x^T via the DMA XBAR, which transposes each 128-column block
            # of the source independently: xt[p, g, t, j, q] = x[g*256 +
            # 2q + t, j*128 + p]
            xt_all = big.tile([P, G, T, DC, P], BF16, tag="xt_all")
            wn_all = big.tile([P, DC, D], F32, tag="wn_all")     # W natural
            wb_all = big.tile([P, DC, D], BF16, tag="wb_all")    # W bf16
            # W^T XBAR dest: wt[p, je, jd, q] = W^T[jd*128+p, je*128+q]
            wt_all = big.tile([P, DC, DC, P], BF16, tag="wt_all")
            mt_all = big.tile([P, DC, D], BF16, tag="mt_all")    # (K^T E)^T
            v_all = big.tile([P, DC, D], BF16, tag="v_all")      # diag(s) W^T
            c_all = big.tile([P, DC, D], BF16, tag="c_all")      # C chunks
            bias_bc = small.tile([P, D], F32, tag="bias_bc")

            b_ap = b_d[:]
            nc.gpsimd.dma_start(
                out=bias_bc,
                in_=bass.AP(tensor=b_ap.tensor, offset=b_ap.offset,
                            ap=[[0, P]] + list(b_ap.ap)),
            )

            # ---- input stream: one in-order sync queue, x2 first (per-t
            # for g0/g1 so the exp stream starts on the first 256KB), then
            # W, then x.  x/W never steal HBM bandwidth from x2.
            x2_tiles = []
            for g in range(G):
                x2_s = x2s.tile([P, T, D], F32, tag="x2_s")
                if g < 2:
                    for t in range(T):
                        nc.sync.dma_start(out=x2_s[:, t, :], in_=x2_t[g][:, t, :])
                else:
                    nc.sync.dma_start(out=x2_s, in_=x2_t[g])
                x2_tiles.append(x2_s)
            nc.sync.dma_start(out=wn_all, in_=w_t)
            x_tiles = []
            for g in range(G):
                x_s = xs.tile([P, T, D], F32, tag="x_s")
                nc.sync.dma_start(out=x_s, in_=x_t[g])
                x_tiles.append(x_s)

            # psum accumulator for M''^T = E^T K: 4 chunks x [128, 512]
            ps_m = psA.tile([P, DC, D], F32, tag="ps_m")

            # ---- EK phase: exp on ACT, scale on DVE, matmuls on PE;
            # GpSimd casts x groups to bf16 as they land.
            for g in range(G):
                x2_s = x2_tiles[g]
                for t in range(T):
                    e_i = e_all[:, g, t, :]
                    rs = stats.tile([P, 1], F32, tag="rs")
                    nc.scalar.activation(
                        out=e_i, in_=x2_s[:, t, :],
                        func=mybir.ActivationFunctionType.Exp,
                        accum_out=rs,
                    )
                    rr = stats.tile([P, 1], F32, tag="rr")
                    nc.vector.reciprocal(out=rr, in_=rs)
                    nc.vector.tensor_scalar_mul(
                        k_all[:, g, t, :], e_i.bitcast(BF16), rr)
                for t in range(T):
                    e_i = e_all[:, g, t, :]
                    k_i = k_all[:, g, t, :]
                    for j in range(DC):
                        nc.tensor.matmul(
                            ps_m[:, j, :],
                            lhsT=e_i[ts(0, P), ts(j, P)],
                            rhs=k_i,
                            start=(g == 0 and t == 0),
                            stop=(g == G - 1 and t == T - 1),
                        )
                nc.gpsimd.tensor_copy(
                    xb_all[:, g, :],
                    x_tiles[g][:].rearrange("p t d -> p (t d)"),
                )

            # W cast rides ACT (idle once the exp stream drains)
            nc.scalar.copy(
                wb_all[:].rearrange("p j d -> p (j d)"),
                wn_all[:].rearrange("p j d -> p (j d)"),
            )

            # ---- XBAR transposes on the sync queue (inputs already issued).
            # x g0/g1 first (first out tiles need them), then W (C phase),
            # then the rest of x.
            def xpose_x(g):
                nc.sync.dma_start_transpose(
                    out=xt_all[:, g], in_=xb_all[:, g, :])

            xpose_x(0)
            xpose_x(1)
            nc.sync.dma_start_transpose(
                out=wt_all, in_=wb_all[:].rearrange("p j d -> p (j d)"))
            for g in range(2, G):
                xpose_x(g)

            # ---- normalize: s = 1/colsum(E); colsum = rowsum of M''^T chunks
            sjs = []
            for j in range(DC):
                cs = stats.tile([P, 1], F32, tag="cs")
                nc.vector.tensor_scalar(
                    out=mt_all[:, j, :], in0=ps_m[:, j, :],
                    scalar1=1.0, scalar2=0.0,
                    op0=mybir.AluOpType.mult,
                    op1=mybir.AluOpType.add,
                    accum_out=cs,
                )
                sj = stats.tile([P, 1], F32, tag="sj")
                nc.vector.reciprocal(out=sj, in_=cs)
                sjs.append(sj)
            for j in range(DC):
                nc.vector.tensor_scalar_mul(
                    v_all[:, j, :], wt_all[:, :, j, :], sjs[j])

            # ---- C = M'' diag(s) W^T  ([D, D])
            for k in range(DC):
                pc = psO.tile([P, D], F32, tag="po")
                for j in range(DC):
                    nc.tensor.matmul(
                        pc,
                        lhsT=mt_all[:, j, ts(k, P)],
                        rhs=v_all[:, j, :],
                        start=(j == 0), stop=(j == DC - 1),
                    )
                if k % 2 == 0:
                    nc.vector.tensor_copy(c_all[:, k, :], pc)
                else:
                    nc.scalar.copy(c_all[:, k, :], pc)

            # ---- out = x @ C + b, streamed per 128-row tile
            for g in range(G):
                og = outp.tile([P, T, D], F32, tag="og")
                for t in range(T):
                    po = psO.tile([P, D], F32, tag="po")
                    for j in range(DC):
                        nc.tensor.matmul(
                            po,
                            lhsT=xt_all[:, g, t, j, :],
                            rhs=c_all[:, j, :],
                            start=(j == 0), stop=(j == DC - 1),
                        )
                    nc.vector.tensor_add(og[:, t, :], po, bias_bc)
                    if g == G - 1:
                        nc.scalar.dma_start(
                            out=out_t[g][:, t, :], in_=og[:, t, :])
                if g < G - 1:
                    nc.scalar.dma_start(out=out_t[g], in_=og)

    nc.compile()
    return nc


def get_nc():
    if "nc" not in _CACHE:
        _CACHE["nc"] = _build_nc()
    return _CACHE["nc"]


def kernel(x, x2, W, b, _trace=False):
    nc = get_nc()
    in_maps = [
        {
            "x": np.ascontiguousarray(x[i], dtype=np.float32),
            "x2": np.ascontiguousarray(x2[i], dtype=np.float32),
            "W": np.ascontiguousarray(W, dtype=np.float32),
            "b": np.ascontiguousarray(b, dtype=np.float32),
        }
        for i in range(B)
    ]
    res = run_bass_kernel_spmd(nc, in_maps, list(range(B)), trace=_trace)
    out = np.stack([res.results[i]["out"] for i in range(B)], axis=0)
    if _trace:
        _CACHE["last_results"] = res
    return out


# revision 11
# speedup vs baseline: 1.1274x; 1.1274x over previous
"""Trainium2 Bass kernel for nn_CrossAttention (efficient-attention form).

Reference computation per batch b:
    K = softmax(x2, axis=-1)           # over D
    Q = softmax(x2, axis=1)            # over N
    out = ((x @ K.T) @ Q) @ W.T + b

Reassociated (matmuls are associative; both softmaxes share exp(x2)):
    E  = exp(x2)                       # one exp serves both softmaxes
    K  = E * (1/rowsum(E))             # per-row scale
    M''^T = E^T K                      # accumulated in PSUM over the x2 stream
    s  = 1/colsum(E) = 1/rowsum(M''^T) # rows of K sum to 1
    C  = M'' diag(s) W^T               # [D, D]
    out = x @ C + b                    # single [N,D]@[D,D] matmul on x

Batch dim B=8 is data-parallel across the 8 cores.

v2 layout/engine plan (from trace analysis of the 75.7us baseline):
  - All transposes (x^T for the final matmul, W^T) moved OFF the tensor
    engine onto the DMA XBAR (dma_start_transpose, bf16): the PE now only
    runs the three real matmul stages (64+16+64 instructions).
  - Inputs stream on ONE sync-engine queue in priority order
    x2 (g0/g1 split per-t for an early exp start) -> W -> x, so x2 never
    loses HBM bandwidth to x; transposes issue on sync after the inputs.
  - fp32->bf16 casts: x on GpSimd (otherwise idle), W on ACT (idle after
    the exp stream). DVE keeps softmax scales, normalize, and bias adds.
  - PSUM: 4 banks for the E^T K accumulator + a 4-buf pool shared by the
    C-phase and out-phase tiles (the baseline had only 2 out banks).
  - Whole M''/C chain in bf16 (measured: bf16 and f32r matmuls stream at
    the same 1 cycle/row, but bf16 enables the DMA-transpose path).
"""

import sys

import numpy as np

if "/opt/trn_rl_repo" not in sys.path:
    sys.path.insert(0, "/opt/trn_rl_repo")

import concourse.bass as bass
import concourse.mybir as mybir
import concourse.tile as tile
from concourse import bacc
from concourse.bass import ds, ts
from concourse.bass_utils import run_bass_kernel_spmd

B, N, D = 8, 2048, 512
P = 128
T = 2             # rows per partition per group
G = N // (P * T)  # 8 row groups
DC = D // P       # 4 column chunks of D
F32 = mybir.dt.float32
BF16 = mybir.dt.bfloat16

_CACHE = {}


def _build_nc():
    nc = bacc.Bacc("TRN2", target_bir_lowering=False, debug=False)
    x_d = nc.declare_dram_parameter("x", [N, D], F32, isOutput=False)
    x2_d = nc.declare_dram_parameter("x2", [N, D], F32, isOutput=False)
    w_d = nc.declare_dram_parameter("W", [D, D], F32, isOutput=False)
    b_d = nc.declare_dram_parameter("b", [D], F32, isOutput=False)
    out_d = nc.declare_dram_parameter("out", [N, D], F32, isOutput=True)

    # input row n = g*256 + p*2 + t -> per-partition DRAM span is 2 rows = 4KB
    x_t = x_d[:].rearrange("(g p t) d -> g p t d", p=P, t=T)
    x2_t = x2_d[:].rearrange("(g p t) d -> g p t d", p=P, t=T)
    w_t = w_d[:].rearrange("(j p) d -> p j d", p=P)
    # the DMA XBAR transpose hands psum partitions back in the same
    # (g p t) row order, so outputs store with the input layout
    out_t = out_d[:].rearrange("(g p t) d -> g p t d", p=P, t=T)

    with tile.TileContext(nc) as tc:
        with (
            tc.tile_pool(name="big", bufs=1) as big,
            tc.tile_pool(name="x2s", bufs=G) as x2s,
            tc.tile_pool(name="xs", bufs=G) as xs,
            tc.tile_pool(name="small", bufs=1) as small,
            tc.tile_pool(name="stats", bufs=4) as stats,
            tc.tile_pool(name="outp", bufs=4) as outp,
            tc.tile_pool(name="psA", bufs=1, space="PSUM") as psA,
            tc.tile_pool(name="psO", bufs=4, space="PSUM") as psO,
        ):
            # ---- persistent SBUF tensors
            e_all = big.tile([P, G, T, D], BF16, tag="e_all")    # exp(x2)
            k_all = big.tile([P, G, T, D], BF16, tag="k_all")    # K rows
            xb_all = big.tile([P, G, T * D], BF16, tag="xb_all")  # x cast
            # x^T via the DMA XBAR, which transposes each 128-column block
            # of the source independently: xt[p, g, t, j, q] = x[g*256 +
            # 2q + t, j*128 + p]
            xt_all = big.tile([P, G, T, DC, P], BF16, tag="xt_all")
            wn_all = big.tile([P, DC, D], F32, tag="wn_all")     # W natural
            wb_all = big.tile([P, DC, D], BF16, tag="wb_all")    # W bf16
            # W^T XBAR dest: wt[p, je, jd, q] = W^T[jd*128+p, je*128+q]
            wt_all = big.tile([P, DC, DC, P], BF16, tag="wt_all")
            mt_all = big.tile([P, DC, D], BF16, tag="mt_all")    # (K^T E)^T
            v_all = big.tile([P, DC, D], BF16, tag="v_all")      # diag(s) W^T
            c_all = big.tile([P, DC, D], BF16, tag="c_all")      # C chunks
            bias_bc = small.tile([P, D], F32, tag="bias_bc")

            b_ap = b_d[:]
            nc.gpsimd.dma_start(
                out=bias_bc,
                in_=bass.AP(tensor=b_ap.tensor, offset=b_ap.offset,
                            ap=[[0, P]] + list(b_ap.ap)),
            )

            # ---- input stream: one in-order sync queue, x2 first (per-t
            # for g0/g1 so the exp stream starts on the first 256KB), with
            # W chunks interleaved mid-stream (needed by the C phase right
            # after the EK stream drains), then x.
            x2_tiles = []
            for g in range(G):
                x2_s = x2s.tile([P, T, D], F32, tag="x2_s")
                if g < 2:
                    for t in range(T):
                        nc.sync.dma_start(out=x2_s[:, t, :], in_=x2_t[g][:, t, :])
                else:
                    nc.sync.dma_start(out=x2_s, in_=x2_t[g])
                x2_tiles.append(x2_s)
                if 2 <= g < 2 + DC:
                    j = g - 2
                    nc.sync.dma_start(out=wn_all[:, j, :], in_=w_t[:, j, :])
            x_tiles = []
            for g in range(G):
                x_s = xs.tile([P, T, D], F32, tag="x_s")
                nc.sync.dma_start(out=x_s, in_=x_t[g])
                x_tiles.append(x_s)

            # psum accumulator for M''^T = E^T K: 4 chunks x [128, 512]
            ps_m = psA.tile([P, DC, D], F32, tag="ps_m")

            # ---- EK phase: exp on ACT, scale on DVE, matmuls on PE;
            # GpSimd casts x groups to bf16 as they land.
            for g in range(G):
                x2_s = x2_tiles[g]
                for t in range(T):
                    e_i = e_all[:, g, t, :]
                    rs = stats.tile([P, 1], F32, tag="rs")
                    nc.scalar.activation(
                        out=e_i, in_=x2_s[:, t, :],
                        func=mybir.ActivationFunctionType.Exp,
                        accum_out=rs,
                    )
                    rr = stats.tile([P, 1], F32, tag="rr")
                    nc.vector.reciprocal(out=rr, in_=rs)
                    nc.vector.tensor_scalar_mul(
                        k_all[:, g, t, :], e_i.bitcast(BF16), rr)
                for t in range(T):
                    e_i = e_all[:, g, t, :]
                    k_i = k_all[:, g, t, :]
                    for j in range(DC):
                        nc.tensor.matmul(
                            ps_m[:, j, :],
                            lhsT=e_i[ts(0, P), ts(j, P)],
                            rhs=k_i,
                            start=(g == 0 and t == 0),
                            stop=(g == G - 1 and t == T - 1),
                        )
            # W cast rides GpSimd (slow there, but GpSimd is otherwise idle
            # and W lands mid-stream, well before it is needed)
            nc.gpsimd.tensor_copy(
                wb_all[:].rearrange("p j d -> p (j d)"),
                wn_all[:].rearrange("p j d -> p (j d)"),
            )
            # W^T via XBAR, issued from the scalar queue right after the
            # exp stream so it never blocks the x transposes on sync
            nc.scalar.dma_start_transpose(
                out=wt_all, in_=wb_all[:].rearrange("p j d -> p (j d)"))

            # x casts ride ACT (idle after the exp stream; GpSimd measured
            # 3.6us per group — far too slow).  First two groups here, the
            # rest interleaved with the out DMAs in the xC loop below.
            def cast_x(g):
                nc.scalar.copy(
                    xb_all[:, g, :],
                    x_tiles[g][:].rearrange("p t d -> p (t d)"),
                )

            # x^T XBAR transposes ride the sync queue (inputs already
            # issued there).  Each must be EMITTED after its group's cast
            # (program order defines the dependency graph).
            def xpose_x(g):
                nc.sync.dma_start_transpose(
                    out=xt_all[:, g], in_=xb_all[:, g, :])

            for g in range(2):
                cast_x(g)
                xpose_x(g)

            # ---- normalize: s = 1/colsum(E); colsum = rowsum of M''^T
            # chunks; per-chunk interleave so the first C matmul unblocks
            # after one chunk, not four.
            for j in range(DC):
                cs = stats.tile([P, 1], F32, tag="cs")
                nc.vector.tensor_scalar(
                    out=mt_all[:, j, :], in0=ps_m[:, j, :],
                    scalar1=1.0, scalar2=0.0,
                    op0=mybir.AluOpType.mult,
                    op1=mybir.AluOpType.add,
                    accum_out=cs,
                )
                sj = stats.tile([P, 1], F32, tag="sj")
                nc.vector.reciprocal(out=sj, in_=cs)
                nc.vector.tensor_scalar_mul(
                    v_all[:, j, :], wt_all[:, :, j, :], sj)

            # ---- C = M'' diag(s) W^T  ([D, D]); psum->sbuf copies on DVE
            for k in range(DC):
                pc = psO.tile([P, D], F32, tag="po")
                for j in range(DC):
                    nc.tensor.matmul(
                        pc,
                        lhsT=mt_all[:, j, ts(k, P)],
                        rhs=v_all[:, j, :],
                        start=(j == 0), stop=(j == DC - 1),
                    )
                nc.vector.tensor_copy(c_all[:, k, :], pc)

            # ---- out = x @ C + b, streamed per 128-row tile
            for g in range(G):
                if g + 2 < G:
                    cast_x(g + 2)
                    xpose_x(g + 2)
                og = outp.tile([P, T, D], F32, tag="og")
                for t in range(T):
                    po = psO.tile([P, D], F32, tag="po")
                    for j in range(DC):
                        nc.tensor.matmul(
                            po,
                            lhsT=xt_all[:, g, t, j, :],
                            rhs=c_all[:, j, :],
                            start=(j == 0), stop=(j == DC - 1),
                        )
                    nc.vector.tensor_add(og[:, t, :], po, bias_bc)
                    if g == G - 1:
                        nc.scalar.dma_start(
                            out=out_t[g][:, t, :], in_=og[:, t, :])
                if g < G - 1:
                    nc.scalar.dma_start(out=out_t[g], in_=og)

    nc.compile()
    return nc


def get_nc():
    if "nc" not in _CACHE:
        _CACHE["nc"] = _build_nc()
    return _CACHE["nc"]


def kernel(x, x2, W, b, _trace=False):
    nc = get_nc()
    in_maps = [
        {
            "x": np.ascontiguousarray(x[i], dtype=np.float32),
            "x2": np.ascontiguousarray(x2[i], dtype=np.float32),
            "W": np.ascontiguousarray(W, dtype=np.float32),
            "b": np.ascontiguousarray(b, dtype=np.float32),
        }
        for i in range(B)
    ]
    res = run_bass_kernel_spmd(nc, in_maps, list(range(B)), trace=_trace)
    out = np.stack([res.results[i]["out"] for i in range(B)], axis=0)
    if _trace:
        _CACHE["last_results"] = res
    return out
